# Initial kernel scaffold
#
"""YOLO-style loss kernel for Trainium2, 8-core data-parallel.

Strategy: shard the 16384 batch across 8 cores (2048 each). Each core
streams its [2048*49, 30] pred/target cells through SBUF in tiles of
128x(CPP*30) fp32, computes every loss term as a masked square
(mask in {0,1} times sqrt-of-weight folded in), concatenates all masked
values into one scratch strip per tile, and uses the scalar engine's
ACTIVATE(Square, accum_out=...) to reduce each tile to a per-partition
partial sum. The host sums the 8x[128,NT] partials and divides by N.

The per-cell math (per cell k, channels [x0,y0,w0,h0,c0, x1..., 20 cls]):
  obj  = t4 > 0, noo = t4 == 0
  noobj term   = 0.5*noo*(d4^2 + d9^2),     d = p - t
  class term   = obj * sum_cls d^2
  iou(i,j) of pred box i vs target box j from xyxy at x/7 +- w/2
  g_j = iou(1,j) > iou(0,j)  (argmax), m_j = max_i iou(i,j)
  conf targets c0 = m1 + g1*(m0-m1), c1 = m0 - g1*(m0-m1)  (last-write-wins)
  resp_0 = obj*(1-g0*g1), resp_1 = obj*(g0+g1-g0*g1)
  contain term = resp_b*(pconf_b - c_b)^2
  loc terms    = 5*resp_b*(dxy^2) and 5*resp_b*(sqrt(pwh+eps)-sqrt(twh+eps))^2
"""

import math

import numpy as np
import concourse.bass as bass
import concourse.tile as tile
from concourse import mybir
from concourse.bass_utils import run_bass_kernel_spmd

F32 = mybir.dt.float32
Alu = mybir.AluOpType
Act = mybir.ActivationFunctionType

# problem constants (hardcoded per harness contract)
BATCH = 16384
S = 7
D = 30
N_CORES = 8
B_PER = BATCH // N_CORES            # 2048
K_CORE = B_PER * S * S              # 100352 cells/core
P = 128
CELLS_PER_PART = K_CORE // P        # 784
NT = 8                              # tiles per core
CPP = CELLS_PER_PART // NT          # 98 cells per partition per tile
INV_S = 1.0 / 7.0
EPS = 1e-6
SQRT5 = math.sqrt(5.0)
SQRT_HALF = math.sqrt(0.5)


def split_sync_waits(nc, max_attached=1):
    """This container's walrus build rejects >1 semaphore wait attached to an
    instruction. Hoist Tile's attached waits into standalone EventSemaphore
    wait instructions (what raw-bass wait_ge emits), which it accepts."""
    n = 0
    for func in nc.m.functions:
        for bb in func.blocks:
            insts = list(bb.instructions)
            out = []
            changed = False
            for inst in insts:
                si = inst.sync_info
                if si is not None and len(si.on_wait) > max_attached:
                    for k, w in enumerate(list(si.on_wait)):
                        wi = mybir.InstEventSemaphore(
                            name=f"{inst.name}-hw{k}", ins=[], outs=[]
                        )
                        wi.engine = inst.engine
                        wi.sync_info = mybir.SyncInfo(on_wait=[w], on_update=[])
                        nc.register_instruction(wi, overwrite=True)
                        out.append(wi)
                        n += 1
                    inst.sync_info = mybir.SyncInfo(
                        on_wait=[], on_update=list(si.on_update)
                    )
                    changed = True
                out.append(inst)
            if changed:
                while len(bb.instructions):
                    bb.instructions.pop()
                for i in out:
                    bb.instructions.append(i)
    return n


def bc(ap, reps):
    """Append a zero-stride broadcast dim of length `reps` to an AP whose
    innermost dim is 1 (drops that singleton)."""
    new = [list(d) for d in ap.ap]
    assert new[-1][1] == 1, new
    new[-1] = [0, reps]
    return bass.AP(tensor=ap.tensor, offset=ap.offset, ap=new)


def build_kernel():
    nc = bass.Bass("TRN2")
    pred = nc.dram_tensor("pred", [K_CORE, D], F32, kind="ExternalInput")
    targ = nc.dram_tensor("targ", [K_CORE, D], F32, kind="ExternalInput")
    out = nc.dram_tensor("out", [P, NT], F32, kind="ExternalOutput")

    # [NT, P, CPP*30] view: tile i, partition p holds CPP contiguous cells
    pred_v = pred.ap().rearrange("(n p c) d -> n p (c d)", n=NT, p=P, c=CPP)
    targ_v = targ.ap().rearrange("(n p c) d -> n p (c d)", n=NT, p=P, c=CPP)

    # scratch strip layout (units of per-cell values, 32 total):
    #   [0:20]  obj-masked class diffs
    #   [20:22] noo-masked conf diffs (ch 4, 9)
    #   [22:24] resp-masked contain diffs
    #   [24:28] sqrt(5)*resp-masked xy diffs
    #   [28:32] sqrt(5)*resp-masked sqrt-wh diffs
    SW = 32

    with tile.TileContext(nc) as tc:
        with (
            tc.tile_pool(name="io", bufs=3) as io,
            tc.tile_pool(name="mid", bufs=2) as mid,
            tc.tile_pool(name="accp", bufs=1) as accp,
        ):
            acc_all = accp.tile([P, NT], F32)

            for it in range(NT):
                pt = io.tile([P, CPP * D], F32, tag="pt")
                tt = io.tile([P, CPP * D], F32, tag="tt")
                nc.sync.dma_start(out=pt[:], in_=pred_v[it])
                nc.sync.dma_start(out=tt[:], in_=targ_v[it])

                p3 = pt[:].rearrange("p (c d) -> p c d", d=D)   # [128,CPP,30]
                t3 = tt[:].rearrange("p (c d) -> p c d", d=D)
                pb = pt[:].rearrange("p (c b f) -> p c b f", b=2 * 3, f=5)[:, :, 0:2, :]
                tb = tt[:].rearrange("p (c b f) -> p c b f", b=2 * 3, f=5)[:, :, 0:2, :]
                # pb/tb: [128, CPP, 2, 5] box view

                scratch = mid.tile([P, CPP, SW], F32, tag="scratch")
                dump = mid.tile([P, CPP * SW], mybir.dt.bfloat16, tag="dump")

                # ---- masks ----
                obj = mid.tile([P, CPP, 1], F32, tag="obj")
                nooh = mid.tile([P, CPP, 1], F32, tag="nooh")
                t4 = t3[:, :, 4:5]
                nc.vector.tensor_scalar(out=obj[:], in0=t4, scalar1=0.0,
                                        scalar2=None, op0=Alu.is_gt)
                nc.vector.tensor_scalar(out=nooh[:], in0=t4, scalar1=0.0,
                                        scalar2=SQRT_HALF, op0=Alu.is_le,
                                        op1=Alu.mult)

                # ---- classes: (p-t)*obj into scratch[0:20] ----
                dcls = mid.tile([P, CPP, 20], F32, tag="dcls")
                nc.vector.tensor_tensor(out=dcls[:], in0=p3[:, :, 10:30],
                                        in1=t3[:, :, 10:30], op=Alu.subtract)
                nc.vector.tensor_tensor(out=scratch[:, :, 0:20], in0=dcls[:],
                                        in1=bc(obj[:], 20), op=Alu.mult)

                # ---- noobj: (p-t)*nooh on conf channels into scratch[20:22] ----
                d49 = mid.tile([P, CPP, 2, 1], F32, tag="d49")
                nc.vector.tensor_tensor(out=d49[:], in0=pb[:, :, :, 4:5],
                                        in1=tb[:, :, :, 4:5], op=Alu.subtract)
                s49 = scratch[:, :, 20:22].rearrange("p c (b o) -> p c b o", o=1)
                nooh2 = bc(nooh[:], 2).rearrange("p c b -> p c b 1")
                nc.vector.tensor_tensor(out=s49, in0=d49[:], in1=nooh2, op=Alu.mult)

                # ---- xyxy for both tensors ----
                # packed [128, CPP, 2(box), 2(ch)]
                uvp = mid.tile([P, CPP, 2, 2], F32, tag="uvp")
                uvt = mid.tile([P, CPP, 2, 2], F32, tag="uvt")
                hwp = mid.tile([P, CPP, 2, 2], F32, tag="hwp")
                hwt = mid.tile([P, CPP, 2, 2], F32, tag="hwt")
                nc.scalar.mul(uvp[:], pb[:, :, :, 0:2], INV_S)
                nc.scalar.mul(uvt[:], tb[:, :, :, 0:2], INV_S)
                nc.scalar.mul(hwp[:], pb[:, :, :, 2:4], 0.5)
                nc.scalar.mul(hwt[:], tb[:, :, :, 2:4], 0.5)

                xy1p = mid.tile([P, CPP, 2, 2], F32, tag="xy1p")
                xy2p = mid.tile([P, CPP, 2, 2], F32, tag="xy2p")
                xy1t = mid.tile([P, CPP, 2, 2], F32, tag="xy1t")
                xy2t = mid.tile([P, CPP, 2, 2], F32, tag="xy2t")
                nc.vector.tensor_tensor(out=xy1p[:], in0=uvp[:], in1=hwp[:], op=Alu.subtract)
                nc.vector.tensor_tensor(out=xy2p[:], in0=uvp[:], in1=hwp[:], op=Alu.add)
                nc.vector.tensor_tensor(out=xy1t[:], in0=uvt[:], in1=hwt[:], op=Alu.subtract)
                nc.vector.tensor_tensor(out=xy2t[:], in0=uvt[:], in1=hwt[:], op=Alu.add)

                # areas (reference-exact: from xyxy differences)
                dxyp = mid.tile([P, CPP, 2, 2], F32, tag="dxyp")
                dxyt = mid.tile([P, CPP, 2, 2], F32, tag="dxyt")
                nc.vector.tensor_tensor(out=dxyp[:], in0=xy2p[:], in1=xy1p[:], op=Alu.subtract)
                nc.vector.tensor_tensor(out=dxyt[:], in0=xy2t[:], in1=xy1t[:], op=Alu.subtract)
                areap = mid.tile([P, CPP, 2, 1], F32, tag="areap")
                areat = mid.tile([P, CPP, 2, 1], F32, tag="areat")
                nc.vector.tensor_tensor(out=areap[:], in0=dxyp[:, :, :, 0:1],
                                        in1=dxyp[:, :, :, 1:2], op=Alu.mult)
                nc.vector.tensor_tensor(out=areat[:], in0=dxyt[:, :, :, 0:1],
                                        in1=dxyt[:, :, :, 1:2], op=Alu.mult)

                # ---- pairwise IoU: loop over target box j; pred boxes vectorized ----
                inter_a = mid.tile([P, CPP, 2, 2], F32, tag="inter")  # [.., j, i]
                union_a = mid.tile([P, CPP, 2, 2], F32, tag="union")
                for j in range(2):
                    xy1tj = bc(xy1t[:, :, j, :].rearrange("p c f -> p c f 1"), 2)\
                        .rearrange("p c f i -> p c i f")
                    xy2tj = bc(xy2t[:, :, j, :].rearrange("p c f -> p c f 1"), 2)\
                        .rearrange("p c f i -> p c i f")
                    lt = mid.tile([P, CPP, 2, 2], F32, tag="lt")
                    rb = mid.tile([P, CPP, 2, 2], F32, tag="rb")
                    nc.vector.tensor_tensor(out=lt[:], in0=xy1p[:], in1=xy1tj, op=Alu.max)
                    nc.vector.tensor_tensor(out=rb[:], in0=xy2p[:], in1=xy2tj, op=Alu.min)
                    whd = mid.tile([P, CPP, 2, 2], F32, tag="whd")
                    nc.vector.tensor_tensor(out=whd[:], in0=rb[:], in1=lt[:], op=Alu.subtract)
                    clip = mid.tile([P, CPP, 2, 2], F32, tag="clip")
                    nc.vector.tensor_scalar(out=clip[:], in0=whd[:], scalar1=0.0,
                                            scalar2=None, op0=Alu.max)
                    nc.vector.tensor_tensor(out=inter_a[:, :, j, :].rearrange("p c i -> p c i 1"),
                                            in0=clip[:, :, :, 0:1],
                                            in1=clip[:, :, :, 1:2], op=Alu.mult)
                    usum = mid.tile([P, CPP, 2, 1], F32, tag="usum")
                    nc.vector.tensor_tensor(out=usum[:], in0=areap[:],
                                            in1=bc(areat[:, :, j, :], 2).rearrange("p c i -> p c i 1"),
                                            op=Alu.add)
                    nc.vector.tensor_tensor(out=union_a[:, :, j, :].rearrange("p c i -> p c i 1"),
                                            in0=usum[:],
                                            in1=inter_a[:, :, j, :].rearrange("p c i -> p c i 1"),
                                            op=Alu.subtract)
                # iou = inter * (1/union)
                rec = mid.tile([P, CPP, 2, 2], F32, tag="rec")
                nc.vector.reciprocal_approx_fast(out=rec[:].rearrange("p c j i -> p (c j i)"),
                                                 in_=union_a[:].rearrange("p c j i -> p (c j i)"))
                iou = mid.tile([P, CPP, 2, 2], F32, tag="iou")
                nc.vector.tensor_tensor(out=iou[:], in0=inter_a[:], in1=rec[:], op=Alu.mult)

                # ---- argmax over pred axis i, per target j ----
                g = mid.tile([P, CPP, 2, 1], F32, tag="g")    # 1.0 if pred box 1 wins
                m = mid.tile([P, CPP, 2, 1], F32, tag="m")    # max iou
                nc.vector.tensor_tensor(out=g[:], in0=iou[:, :, :, 1:2],
                                        in1=iou[:, :, :, 0:1], op=Alu.is_gt)
                nc.vector.tensor_tensor(out=m[:], in0=iou[:, :, :, 1:2],
                                        in1=iou[:, :, :, 0:1], op=Alu.max)

                # ---- conf targets (last-write-wins) ----
                m0 = m[:, :, 0, :]
                m1 = m[:, :, 1, :]
                g0 = g[:, :, 0, :]
                g1 = g[:, :, 1, :]
                dm = mid.tile([P, CPP, 1], F32, tag="dm")
                gdm = mid.tile([P, CPP, 1], F32, tag="gdm")
                ct = mid.tile([P, CPP, 2, 1], F32, tag="ct")
                nc.vector.tensor_tensor(out=dm[:], in0=m0, in1=m1, op=Alu.subtract)
                nc.vector.tensor_tensor(out=gdm[:], in0=g1, in1=dm[:], op=Alu.mult)
                nc.vector.tensor_tensor(out=ct[:, :, 0, :], in0=m1, in1=gdm[:], op=Alu.add)
                nc.vector.tensor_tensor(out=ct[:, :, 1, :], in0=m0, in1=gdm[:], op=Alu.subtract)

                # ---- responsibility masks ----
                gg = mid.tile([P, CPP, 1], F32, tag="gg")
                s01 = mid.tile([P, CPP, 1], F32, tag="s01")
                rr = mid.tile([P, CPP, 2, 1], F32, tag="rr")
                nc.vector.tensor_tensor(out=gg[:], in0=g0, in1=g1, op=Alu.mult)
                nc.vector.tensor_tensor(out=s01[:], in0=g0, in1=g1, op=Alu.add)
                nc.vector.tensor_scalar(out=rr[:, :, 0, :], in0=gg[:], scalar1=-1.0,
                                        scalar2=1.0, op0=Alu.mult, op1=Alu.add)
                nc.vector.scalar_tensor_tensor(out=rr[:, :, 1, :], in0=gg[:], scalar=-1.0,
                                               in1=s01[:], op0=Alu.mult, op1=Alu.add)
                rm = mid.tile([P, CPP, 2, 1], F32, tag="rm")
                objb = bc(obj[:], 2).rearrange("p c b -> p c b 1")
                nc.vector.tensor_tensor(out=rm[:], in0=rr[:], in1=objb, op=Alu.mult)
                rm5 = mid.tile([P, CPP, 2, 1], F32, tag="rm5")
                nc.vector.tensor_scalar(out=rm5[:], in0=rm[:], scalar1=SQRT5,
                                        scalar2=None, op0=Alu.mult)

                # ---- contain: (pconf - ct)*rm into scratch[22:24] ----
                e = mid.tile([P, CPP, 2, 1], F32, tag="e")
                nc.vector.tensor_tensor(out=e[:], in0=pb[:, :, :, 4:5], in1=ct[:], op=Alu.subtract)
                sct = scratch[:, :, 22:24].rearrange("p c (b o) -> p c b o", o=1)
                nc.vector.tensor_tensor(out=sct, in0=e[:], in1=rm[:], op=Alu.mult)

                # ---- loc xy: (pxy - txy)*rm5 into scratch[24:28] ----
                dxy = mid.tile([P, CPP, 2, 2], F32, tag="dxy")
                nc.vector.tensor_tensor(out=dxy[:], in0=pb[:, :, :, 0:2],
                                        in1=tb[:, :, :, 0:2], op=Alu.subtract)
                sxy = scratch[:, :, 24:28].rearrange("p c (b f) -> p c b f", b=2)
                nc.vector.tensor_tensor(out=sxy, in0=dxy[:], in1=bc(rm5[:], 2), op=Alu.mult)

                # ---- loc wh: (sqrt(pwh+eps) - sqrt(twh+eps))*rm5 into scratch[28:32] ----
                sqp = mid.tile([P, CPP, 2, 2], F32, tag="sqp")
                sqt = mid.tile([P, CPP, 2, 2], F32, tag="sqt")
                nc.scalar.activation(out=sqp[:], in_=pb[:, :, :, 2:4], func=Act.Sqrt,
                                     bias=EPS, scale=1.0)
                nc.scalar.activation(out=sqt[:], in_=tb[:, :, :, 2:4], func=Act.Sqrt,
                                     bias=EPS, scale=1.0)
                dwh = mid.tile([P, CPP, 2, 2], F32, tag="dwh")
                nc.vector.tensor_tensor(out=dwh[:], in0=sqp[:], in1=sqt[:], op=Alu.subtract)
                swh = scratch[:, :, 28:32].rearrange("p c (b f) -> p c b f", b=2)
                nc.vector.tensor_tensor(out=swh, in0=dwh[:], in1=bc(rm5[:], 2), op=Alu.mult)

                # ---- one fused square+sum over the whole strip ----
                nc.scalar.activation(out=dump[:], in_=scratch[:].rearrange("p c w -> p (c w)"),
                                     func=Act.Square, accum_out=acc_all[:, it:it + 1])

            nc.sync.dma_start(out=out[:], in_=acc_all[:])

    split_sync_waits(nc)
    return nc


_NC_CACHE = None


def kernel(pred_tensor: np.ndarray, target_tensor: np.ndarray) -> np.ndarray:
    global _NC_CACHE
    if _NC_CACHE is None:
        _NC_CACHE = build_kernel()
    nc = _NC_CACHE

    p = np.ascontiguousarray(pred_tensor, dtype=np.float32).reshape(N_CORES, K_CORE, D)
    t = np.ascontiguousarray(target_tensor, dtype=np.float32).reshape(N_CORES, K_CORE, D)
    in_maps = [{"pred": p[i], "targ": t[i]} for i in range(N_CORES)]
    res = run_bass_kernel_spmd(nc, in_maps, core_ids=list(range(N_CORES)))
    total = 0.0
    for i in range(N_CORES):
        total += res.results[i]["out"].astype(np.float64).sum()
    return np.float32(total / BATCH)


# revision 12
# speedup vs baseline: 1.5543x; 1.5543x over previous
"""YOLO-style loss kernel for Trainium2, 8-core data-parallel.

Strategy: shard the 16384 batch across 8 cores (2048 each). Each core
streams its [2048*49, 30] pred/target cells through SBUF in tiles of
128x(CPP*30) fp32, computes every loss term as a masked square
(mask in {0,1} times sqrt-of-weight folded in), concatenates all masked
values into one scratch strip per tile, and uses the scalar engine's
ACTIVATE(Square, accum_out=...) to reduce each tile to a per-partition
partial sum. The host sums the 8x[128,NT] partials and divides by N.

The per-cell math (per cell k, channels [x0,y0,w0,h0,c0, x1..., 20 cls]):
  obj  = t4 > 0, noo = t4 == 0
  noobj term   = 0.5*noo*(d4^2 + d9^2),     d = p - t
  class term   = obj * sum_cls d^2
  iou(i,j) of pred box i vs target box j from xyxy at x/7 +- w/2
  g_j = iou(1,j) > iou(0,j)  (argmax), m_j = max_i iou(i,j)
  conf targets c0 = m1 + g1*(m0-m1), c1 = m0 - g1*(m0-m1)  (last-write-wins)
  resp_0 = obj*(1-g0*g1), resp_1 = obj*(g0+g1-g0*g1)
  contain term = resp_b*(pconf_b - c_b)^2
  loc terms    = 5*resp_b*(dxy^2) and 5*resp_b*(sqrt(pwh+eps)-sqrt(twh+eps))^2
"""

import math

import numpy as np
import concourse.bass as bass
import concourse.tile as tile
from concourse import mybir
from concourse.bass_utils import run_bass_kernel_spmd

F32 = mybir.dt.float32
Alu = mybir.AluOpType
Act = mybir.ActivationFunctionType

# problem constants (hardcoded per harness contract)
BATCH = 16384
S = 7
D = 30
N_CORES = 8
B_PER = BATCH // N_CORES            # 2048
K_CORE = B_PER * S * S              # 100352 cells/core
P = 128
CELLS_PER_PART = K_CORE // P        # 784
NT = 8                              # tiles per core
CPP = CELLS_PER_PART // NT          # 98 cells per partition per tile
INV_S = 1.0 / 7.0
EPS = 1e-6
SQRT5 = math.sqrt(5.0)
SQRT_HALF = math.sqrt(0.5)


def split_sync_waits(nc, max_attached=1):
    """This container's walrus build rejects >1 semaphore wait attached to an
    instruction. Hoist Tile's attached waits into standalone EventSemaphore
    wait instructions (what raw-bass wait_ge emits), which it accepts."""
    n = 0
    for func in nc.m.functions:
        for bb in func.blocks:
            insts = list(bb.instructions)
            out = []
            changed = False
            for inst in insts:
                si = inst.sync_info
                if si is not None and len(si.on_wait) > max_attached:
                    for k, w in enumerate(list(si.on_wait)):
                        wi = mybir.InstEventSemaphore(
                            name=f"{inst.name}-hw{k}", ins=[], outs=[]
                        )
                        wi.engine = inst.engine
                        wi.sync_info = mybir.SyncInfo(on_wait=[w], on_update=[])
                        nc.register_instruction(wi, overwrite=True)
                        out.append(wi)
                        n += 1
                    inst.sync_info = mybir.SyncInfo(
                        on_wait=[], on_update=list(si.on_update)
                    )
                    changed = True
                out.append(inst)
            if changed:
                while len(bb.instructions):
                    bb.instructions.pop()
                for i in out:
                    bb.instructions.append(i)
    return n


def bc(ap, reps):
    """Replace a trailing singleton dim with a zero-stride broadcast dim."""
    new = [list(d) for d in ap.ap]
    assert new[-1][1] == 1, new
    new[-1] = [0, reps]
    return bass.AP(tensor=ap.tensor, offset=ap.offset, ap=new)


def d1(ap):
    """Drop a trailing singleton dim."""
    new = [list(d) for d in ap.ap]
    assert new[-1][1] == 1, new
    return bass.AP(tensor=ap.tensor, offset=ap.offset, ap=new[:-1])


def abc(ap, reps):
    """Append a zero-stride broadcast dim."""
    new = [list(d) for d in ap.ap] + [[0, reps]]
    return bass.AP(tensor=ap.tensor, offset=ap.offset, ap=new)


def ibc(ap, pos, reps):
    """Insert a zero-stride broadcast dim at ap-list position pos."""
    new = [list(d) for d in ap.ap]
    new.insert(pos, [0, reps])
    return bass.AP(tensor=ap.tensor, offset=ap.offset, ap=new)


def build_kernel(repeat=1):
    nc = bass.Bass("TRN2")
    pred = nc.dram_tensor("pred", [K_CORE, D], F32, kind="ExternalInput")
    targ = nc.dram_tensor("targ", [K_CORE, D], F32, kind="ExternalInput")
    out = nc.dram_tensor("out", [P, NT * repeat], F32, kind="ExternalOutput")

    # [NT, P, CPP*30] view: tile i, partition p holds CPP contiguous cells
    pred_v = pred.ap().rearrange("(n p c) d -> n p (c d)", n=NT, p=P, c=CPP)
    targ_v = targ.ap().rearrange("(n p c) d -> n p (c d)", n=NT, p=P, c=CPP)

    # scratch strip layout (units of per-cell values, 32 total):
    #   [0:20]  obj-masked class diffs
    #   [20:22] noo-masked conf diffs (ch 4, 9)
    #   [22:24] resp-masked contain diffs
    #   [24:28] sqrt(5)*resp-masked xy diffs
    #   [28:32] sqrt(5)*resp-masked sqrt-wh diffs
    SW = 32

    with tile.TileContext(nc) as tc:
        with (
            tc.tile_pool(name="io", bufs=3) as io,
            tc.tile_pool(name="mid", bufs=1) as mid,
            tc.tile_pool(name="strip", bufs=2) as strip,
            tc.tile_pool(name="dumpp", bufs=1) as dumpp,
            tc.tile_pool(name="accp", bufs=1) as accp,
        ):
            acc_all = accp.tile([P, NT * repeat], F32)
            eps_t = accp.tile([P, 1], F32)
            nc.vector.memset(eps_t[:], EPS)

            for rit in range(NT * repeat):
                it = rit % NT
                pt = io.tile([P, CPP * D], F32, tag="pt")
                tt = io.tile([P, CPP * D], F32, tag="tt")
                nc.sync.dma_start(out=pt[:], in_=pred_v[it])
                nc.sync.dma_start(out=tt[:], in_=targ_v[it])

                p3 = pt[:].rearrange("p (c d) -> p c d", d=D)   # [128,CPP,30]
                t3 = tt[:].rearrange("p (c d) -> p c d", d=D)
                pb = pt[:].rearrange("p (c b f) -> p c b f", b=2 * 3, f=5)[:, :, 0:2, :]
                tb = tt[:].rearrange("p (c b f) -> p c b f", b=2 * 3, f=5)[:, :, 0:2, :]
                # pb/tb: [128, CPP, 2, 5] box view

                scratch = strip.tile([P, CPP, SW], F32, tag="scratch")
                dump = dumpp.tile([P, CPP * SW], mybir.dt.bfloat16, tag="dump")

                # ---- masks ----
                obj = mid.tile([P, CPP, 1], F32, tag="obj")
                nooh = mid.tile([P, CPP, 1], F32, tag="nooh")
                t4 = t3[:, :, 4:5]
                nc.vector.tensor_scalar(out=obj[:], in0=t4, scalar1=0.0,
                                        scalar2=None, op0=Alu.is_gt)
                nc.vector.tensor_scalar(out=nooh[:], in0=t4, scalar1=0.0,
                                        scalar2=SQRT_HALF, op0=Alu.is_le,
                                        op1=Alu.mult)

                # ---- classes: (p-t)*obj into scratch[0:20] ----
                dcls = mid.tile([P, CPP, 20], F32, tag="dcls")
                nc.vector.tensor_tensor(out=dcls[:], in0=p3[:, :, 10:30],
                                        in1=t3[:, :, 10:30], op=Alu.subtract)
                nc.vector.tensor_tensor(out=scratch[:, :, 0:20], in0=dcls[:],
                                        in1=bc(obj[:], 20), op=Alu.mult)

                # ---- noobj: (p-t)*nooh on conf channels into scratch[20:22] ----
                d49 = mid.tile([P, CPP, 2], F32, tag="d49")
                nc.vector.tensor_tensor(out=d49[:], in0=d1(pb[:, :, :, 4:5]),
                                        in1=d1(tb[:, :, :, 4:5]), op=Alu.subtract)
                nc.vector.tensor_tensor(out=scratch[:, :, 20:22], in0=d49[:],
                                        in1=bc(nooh[:], 2), op=Alu.mult)

                # ---- xyxy for both tensors ----
                # packed [128, CPP, 2(box), 2(ch)]
                uvp = mid.tile([P, CPP, 2, 2], F32, tag="uvp")
                uvt = mid.tile([P, CPP, 2, 2], F32, tag="uvt")
                hwp = mid.tile([P, CPP, 2, 2], F32, tag="hwp")
                hwt = mid.tile([P, CPP, 2, 2], F32, tag="hwt")
                nc.scalar.mul(uvp[:], pb[:, :, :, 0:2], INV_S)
                nc.scalar.mul(uvt[:], tb[:, :, :, 0:2], INV_S)
                nc.scalar.mul(hwp[:], pb[:, :, :, 2:4], 0.5)
                nc.scalar.mul(hwt[:], tb[:, :, :, 2:4], 0.5)

                xy1p = mid.tile([P, CPP, 2, 2], F32, tag="xy1p")
                xy2p = mid.tile([P, CPP, 2, 2], F32, tag="xy2p")
                xy1t = mid.tile([P, CPP, 2, 2], F32, tag="xy1t")
                xy2t = mid.tile([P, CPP, 2, 2], F32, tag="xy2t")
                nc.vector.tensor_tensor(out=xy1p[:], in0=uvp[:], in1=hwp[:], op=Alu.subtract)
                nc.vector.tensor_tensor(out=xy2p[:], in0=uvp[:], in1=hwp[:], op=Alu.add)
                nc.vector.tensor_tensor(out=xy1t[:], in0=uvt[:], in1=hwt[:], op=Alu.subtract)
                nc.vector.tensor_tensor(out=xy2t[:], in0=uvt[:], in1=hwt[:], op=Alu.add)

                # areas (reference-exact: from xyxy differences)
                dxyp = mid.tile([P, CPP, 2, 2], F32, tag="dxyp")
                dxyt = mid.tile([P, CPP, 2, 2], F32, tag="dxyt")
                nc.vector.tensor_tensor(out=dxyp[:], in0=xy2p[:], in1=xy1p[:], op=Alu.subtract)
                nc.vector.tensor_tensor(out=dxyt[:], in0=xy2t[:], in1=xy1t[:], op=Alu.subtract)
                areap = mid.tile([P, CPP, 2], F32, tag="areap")
                areat = mid.tile([P, CPP, 2], F32, tag="areat")
                nc.vector.tensor_tensor(out=areap[:], in0=d1(dxyp[:, :, :, 0:1]),
                                        in1=d1(dxyp[:, :, :, 1:2]), op=Alu.mult)
                nc.vector.tensor_tensor(out=areat[:], in0=d1(dxyt[:, :, :, 0:1]),
                                        in1=d1(dxyt[:, :, :, 1:2]), op=Alu.mult)

                # ---- pairwise IoU: loop over target box j; pred boxes vectorized ----
                inter_a = mid.tile([P, CPP, 2, 2], F32, tag="inter")  # [.., j, i]
                union_a = mid.tile([P, CPP, 2, 2], F32, tag="union")
                for j in range(2):
                    # target box j broadcast over pred axis i: [P, CPP, 2(i), 2(ch)]
                    xy1tj = ibc(xy1t[:, :, j, :], 2, 2)
                    xy2tj = ibc(xy2t[:, :, j, :], 2, 2)
                    lt = mid.tile([P, CPP, 2, 2], F32, tag="lt")
                    rb = mid.tile([P, CPP, 2, 2], F32, tag="rb")
                    nc.vector.tensor_tensor(out=lt[:], in0=xy1p[:], in1=xy1tj, op=Alu.max)
                    nc.vector.tensor_tensor(out=rb[:], in0=xy2p[:], in1=xy2tj, op=Alu.min)
                    whd = mid.tile([P, CPP, 2, 2], F32, tag="whd")
                    nc.vector.tensor_tensor(out=whd[:], in0=rb[:], in1=lt[:], op=Alu.subtract)
                    clip = mid.tile([P, CPP, 2, 2], F32, tag="clip")
                    nc.vector.tensor_scalar(out=clip[:], in0=whd[:], scalar1=0.0,
                                            scalar2=None, op0=Alu.max)
                    nc.vector.tensor_tensor(out=inter_a[:, :, j, :],
                                            in0=d1(clip[:, :, :, 0:1]),
                                            in1=d1(clip[:, :, :, 1:2]), op=Alu.mult)
                    usum = mid.tile([P, CPP, 2], F32, tag="usum")
                    nc.vector.tensor_tensor(out=usum[:], in0=areap[:],
                                            in1=bc(areat[:, :, j:j + 1], 2),
                                            op=Alu.add)
                    nc.vector.tensor_tensor(out=union_a[:, :, j, :],
                                            in0=usum[:],
                                            in1=inter_a[:, :, j, :],
                                            op=Alu.subtract)
                # iou = inter * (1/union)
                rec = mid.tile([P, CPP, 2, 2], F32, tag="rec")
                nc.vector.reciprocal(out=rec[:].rearrange("p c j i -> p (c j i)"),
                                     in_=union_a[:].rearrange("p c j i -> p (c j i)"))
                iou = mid.tile([P, CPP, 2, 2], F32, tag="iou")
                nc.vector.tensor_tensor(out=iou[:], in0=inter_a[:], in1=rec[:], op=Alu.mult)

                # ---- argmax over pred axis i, per target j ----
                g = mid.tile([P, CPP, 2], F32, tag="g")    # 1.0 if pred box 1 wins
                m = mid.tile([P, CPP, 2], F32, tag="m")    # max iou
                nc.vector.tensor_tensor(out=g[:], in0=d1(iou[:, :, :, 1:2]),
                                        in1=d1(iou[:, :, :, 0:1]), op=Alu.is_gt)
                nc.vector.tensor_tensor(out=m[:], in0=d1(iou[:, :, :, 1:2]),
                                        in1=d1(iou[:, :, :, 0:1]), op=Alu.max)

                # ---- conf targets (last-write-wins) ----
                m0 = m[:, :, 0:1]
                m1 = m[:, :, 1:2]
                g0 = g[:, :, 0:1]
                g1 = g[:, :, 1:2]
                dm = mid.tile([P, CPP, 1], F32, tag="dm")
                gdm = mid.tile([P, CPP, 1], F32, tag="gdm")
                ct = mid.tile([P, CPP, 2], F32, tag="ct")
                nc.vector.tensor_tensor(out=dm[:], in0=m0, in1=m1, op=Alu.subtract)
                nc.vector.tensor_tensor(out=gdm[:], in0=g1, in1=dm[:], op=Alu.mult)
                nc.vector.tensor_tensor(out=ct[:, :, 0:1], in0=m1, in1=gdm[:], op=Alu.add)
                nc.vector.tensor_tensor(out=ct[:, :, 1:2], in0=m0, in1=gdm[:], op=Alu.subtract)

                # ---- responsibility masks ----
                gg = mid.tile([P, CPP, 1], F32, tag="gg")
                s01 = mid.tile([P, CPP, 1], F32, tag="s01")
                rr = mid.tile([P, CPP, 2], F32, tag="rr")
                nc.vector.tensor_tensor(out=gg[:], in0=g0, in1=g1, op=Alu.mult)
                nc.vector.tensor_tensor(out=s01[:], in0=g0, in1=g1, op=Alu.add)
                nc.vector.tensor_scalar(out=rr[:, :, 0:1], in0=gg[:], scalar1=-1.0,
                                        scalar2=1.0, op0=Alu.mult, op1=Alu.add)
                nc.vector.scalar_tensor_tensor(out=rr[:, :, 1:2], in0=gg[:], scalar=-1.0,
                                               in1=s01[:], op0=Alu.mult, op1=Alu.add)
                rm = mid.tile([P, CPP, 2], F32, tag="rm")
                nc.vector.tensor_tensor(out=rm[:], in0=rr[:], in1=bc(obj[:], 2), op=Alu.mult)
                rm5 = mid.tile([P, CPP, 2], F32, tag="rm5")
                nc.vector.tensor_scalar(out=rm5[:], in0=rm[:], scalar1=SQRT5,
                                        scalar2=None, op0=Alu.mult)

                # ---- contain: (pconf - ct)*rm into scratch[22:24] ----
                e = mid.tile([P, CPP, 2], F32, tag="e")
                nc.vector.tensor_tensor(out=e[:], in0=d1(pb[:, :, :, 4:5]), in1=ct[:], op=Alu.subtract)
                nc.vector.tensor_tensor(out=scratch[:, :, 22:24], in0=e[:], in1=rm[:], op=Alu.mult)

                # ---- loc xy: (pxy - txy)*rm5 into scratch[24:28] ----
                dxy = mid.tile([P, CPP, 2, 2], F32, tag="dxy")
                nc.vector.tensor_tensor(out=dxy[:], in0=pb[:, :, :, 0:2],
                                        in1=tb[:, :, :, 0:2], op=Alu.subtract)
                sxy = scratch[:, :, 24:28].rearrange("p c (b f) -> p c b f", b=2)
                nc.vector.tensor_tensor(out=sxy, in0=dxy[:], in1=abc(rm5[:], 2), op=Alu.mult)

                # ---- loc wh: (sqrt(pwh+eps) - sqrt(twh+eps))*rm5 into scratch[28:32] ----
                sqp = mid.tile([P, CPP, 2, 2], F32, tag="sqp")
                sqt = mid.tile([P, CPP, 2, 2], F32, tag="sqt")
                nc.scalar.activation(out=sqp[:], in_=pb[:, :, :, 2:4], func=Act.Sqrt,
                                     bias=eps_t[:], scale=1.0)
                nc.scalar.activation(out=sqt[:], in_=tb[:, :, :, 2:4], func=Act.Sqrt,
                                     bias=eps_t[:], scale=1.0)
                dwh = mid.tile([P, CPP, 2, 2], F32, tag="dwh")
                nc.vector.tensor_tensor(out=dwh[:], in0=sqp[:], in1=sqt[:], op=Alu.subtract)
                swh = scratch[:, :, 28:32].rearrange("p c (b f) -> p c b f", b=2)
                nc.vector.tensor_tensor(out=swh, in0=dwh[:], in1=abc(rm5[:], 2), op=Alu.mult)

                # ---- one fused square+sum over the whole strip ----
                nc.scalar.activation(out=dump[:], in_=scratch[:].rearrange("p c w -> p (c w)"),
                                     func=Act.Square, accum_out=acc_all[:, rit:rit + 1])

            nc.sync.dma_start(out=out[:], in_=acc_all[:])

    split_sync_waits(nc)
    return nc


_NC_CACHE = None


def kernel(pred_tensor: np.ndarray, target_tensor: np.ndarray) -> np.ndarray:
    global _NC_CACHE
    if _NC_CACHE is None:
        _NC_CACHE = build_kernel()
    nc = _NC_CACHE

    p = np.ascontiguousarray(pred_tensor, dtype=np.float32).reshape(N_CORES, K_CORE, D)
    t = np.ascontiguousarray(target_tensor, dtype=np.float32).reshape(N_CORES, K_CORE, D)
    in_maps = [{"pred": p[i], "targ": t[i]} for i in range(N_CORES)]
    res = run_bass_kernel_spmd(nc, in_maps, core_ids=list(range(N_CORES)))
    total = 0.0
    for i in range(N_CORES):
        total += res.results[i]["out"].astype(np.float64).sum()
    return np.float32(total / BATCH)
